# revision 20
# baseline (speedup 1.0000x reference)
"""Trainium2 Bass kernel for AttentionWithRoPE (B=2, S=2048, HID=2048, H=16, D=128).

Sharding (8 cores): tensor-parallel over heads x data-parallel over batch.
Core c handles batch c//4 and heads 4*(c%4) .. 4*(c%4)+4:
  - QKV^T projections as fp32r PE matmuls with the HID contraction on
    partitions (hidden arrives pre-transposed from the host). Q pass, K pass,
    then V pass; Q^T/K^T/V stay resident in SBUF (a 2-slot rotating pool
    hosts wq/wk -> wv -> A^T over time, so each phase's loads prefetch
    during the previous one). RoPE is fused on the DVE reading the
    projection PSUM directly (partition-shifted reads are legal vs PSUM).
  - Causal attention per head in scores^T orientation ([k, q]: the PV
    contraction dim k lands on partitions, so P^T feeds the PE directly and
    softmax needs no transposes). exp on ScalarE straight from PSUM with the
    1/sqrt(D) scale folded in; fully-masked k-blocks are skipped; diagonal
    blocks multiply a 0/1 upper-tri mask; the softmax denominator is a
    ones-vector PE matmul accumulated alongside PV; normalization is
    reciprocal + K=1 PE-matmul broadcast + DVE multiply.
  - Output projection partial with the core's w_o column slice; the host
    sums the four partials per batch (the TP reduce).
All matmul operands are float32r (TF32-like: full PE rate at moving
free-dim >= 256, ~1.5e-4 rel err); end-to-end output error vs the fp32
reference is ~2e-4. Big DMAs are chunked so consumers start on the first
chunk; small DMAs are merged to amortize descriptor cost.
"""
import numpy as np
from contextlib import ExitStack

import concourse.bass as bass
import concourse.tile as tile
from concourse import bacc, mybir
from concourse.bass_utils import run_bass_kernel_spmd

B, S, HID = 2, 2048, 2048
H, D = 16, 128
NCORES = 8
NH = 4                 # heads per core
HC = HID // 128        # hid chunks
ST = 256               # phase-A s-tile width
NST = S // ST
QT = 512               # phase-B q-tile width
NQT = S // QT
DSCALE = float(D) ** -0.5
F32 = mybir.dt.float32
F32R = mybir.dt.float32r

_CACHED = {}


def _build_nc():
    nc = bacc.Bacc("TRN2", target_bir_lowering=False, debug=False,
                   num_devices=NCORES)
    hT = nc.dram_tensor("hT", [HID, S], F32R, kind="ExternalInput")
    wqT = nc.dram_tensor("wqT", [HID, NH * D], F32R, kind="ExternalInput")
    wkT = nc.dram_tensor("wkT", [HID, NH * D], F32R, kind="ExternalInput")
    wvT = nc.dram_tensor("wvT", [HID, NH * D], F32R, kind="ExternalInput")
    woT = nc.dram_tensor("woT", [NH * D, HID], F32R, kind="ExternalInput")
    cosT = nc.dram_tensor("cosT", [D, S], F32, kind="ExternalInput")
    sinS = nc.dram_tensor("sinS", [D, S], F32, kind="ExternalInput")
    tri = nc.dram_tensor("tri", [128, 128], F32, kind="ExternalInput")
    ones = nc.dram_tensor("ones", [128, 1], F32R, kind="ExternalInput")
    onesr = nc.dram_tensor("onesr", [1, 128], F32R, kind="ExternalInput")
    out = nc.dram_tensor("out", [S, HID], F32, kind="ExternalOutput")

    hT_r = hT.ap().rearrange("(hc p) s -> p hc s", p=128)
    wqT_r = wqT.ap().rearrange("(hc p) m -> p hc m", p=128)
    wkT_r = wkT.ap().rearrange("(hc p) m -> p hc m", p=128)
    wvT_r = wvT.ap().rearrange("(hc p) m -> p hc m", p=128)
    woT_r = woT.ap().rearrange("(g p) n -> p g n", p=128)

    AST = 512              # phase-A s-tile width (N of the QK-pass matmuls)
    ANST = S // AST

    with tile.TileContext(nc) as tc, ExitStack() as ctx:
        # ---- small constants ----
        constp = ctx.enter_context(tc.tile_pool(name="const", bufs=1))
        tri_sb = constp.tile([128, 128], F32, tag="tri", name="tri")
        nc.sync.dma_start(tri_sb[:], tri.ap())
        ones_sb = constp.tile([128, 1], F32R, tag="ones", name="ones")
        nc.sync.dma_start(ones_sb[:], ones.ap())
        onesr_sb = constp.tile([1, 128], F32R, tag="onesr", name="onesr")
        nc.sync.dma_start(onesr_sb[:], onesr.ap())

        # Q^T/K^T stay resident in SBUF through attention
        qkp = ctx.enter_context(tc.tile_pool(name="qk", bufs=1))
        qsb = qkp.tile([128, NH, S], F32R, tag="qsb", name="qsb")
        ksb = qkp.tile([128, NH, S], F32R, tag="ksb", name="ksb")

        # Rotating 2-slot pool (32KB/partition each) hosting, over time:
        #   slot0: wq -> wv -> at   slot1: wk -> v_sb
        # Tile's WAR tracking turns each reuse into a prefetch window.
        wpool = ctx.enter_context(tc.tile_pool(name="aw", bufs=2))
        wq_sb = wpool.tile([128, HC, NH * D], F32R, tag="w", name="wq")
        wk_sb = wpool.tile([128, HC, NH * D], F32R, tag="w", name="wk")
        for c in range(4):
            h4 = slice(4 * c, 4 * c + 4)
            nc.sync.dma_start(wq_sb[:, h4, :], wqT_r[:, h4, :])
        for c in range(4):
            h4 = slice(4 * c, 4 * c + 4)
            nc.sync.dma_start(wk_sb[:, h4, :], wkT_r[:, h4, :])

        # ================= Phase A: Q pass, K pass =================
        with ExitStack() as astack:
            hpool = astack.enter_context(tc.tile_pool(name="ah", bufs=3))
            cspool = astack.enter_context(tc.tile_pool(name="acs", bufs=1))
            ropep = astack.enter_context(tc.tile_pool(name="arope", bufs=1))
            psA = astack.enter_context(
                tc.tile_pool(name="apsqk", bufs=3, space="PSUM"))

            for st in range(ANST):
                sl = bass.ts(st, AST)
                hb = [hpool.tile([128, HC // 2, AST], F32R, tag="h",
                                 name=f"hb{half}") for half in range(2)]
                for half in range(2):
                    for c in range(4):
                        nc.sync.dma_start(
                            hb[half][:, 2 * c:2 * c + 2, :],
                            hT_r[:, slice(8 * half + 2 * c,
                                          8 * half + 2 * c + 2), sl])
                cs_t = cspool.tile([128, AST], F32, tag="cs", name="cs")
                nc.sync.dma_start(cs_t[:], cosT.ap()[:, sl])
                ss_t = cspool.tile([128, AST], F32, tag="ss", name="ss")
                nc.sync.dma_start(ss_t[:], sinS.ap()[:, sl])
                for wsb, dsb in ((wq_sb, qsb), (wk_sb, ksb)):
                    for h in range(NH):
                        ps = psA.tile([128, AST], F32, tag="psqk",
                                      name="psqk")
                        for hc in range(HC):
                            nc.tensor.matmul(
                                ps[:],
                                wsb[:, hc, h * D:(h + 1) * D],
                                hb[hc // 8][:, hc % 8, :],
                                start=(hc == 0), stop=(hc == HC - 1),
                            )
                        # RoPE: out = x*cos + shift(x)*sin_signed. The
                        # partition-shifted reads go straight to PSUM (walrus
                        # requires equal base partitions only when BOTH
                        # operands are in SBUF).
                        tsin = ropep.tile([128, AST], F32, tag="tsin",
                                          name="tsin")
                        nc.vector.tensor_tensor(
                            tsin[0:64, :], ps[64:128, :], ss_t[0:64, :],
                            mybir.AluOpType.mult)
                        nc.vector.tensor_tensor(
                            tsin[64:128, :], ps[0:64, :], ss_t[64:128, :],
                            mybir.AluOpType.mult)
                        tcos = ropep.tile([128, AST], F32, tag="tcos",
                                          name="tcos")
                        nc.vector.tensor_tensor(
                            tcos[:], ps[:], cs_t[:], mybir.AluOpType.mult)
                        nc.vector.tensor_tensor(
                            dsb[:, h, sl], tcos[:], tsin[:],
                            mybir.AluOpType.add)

            # wv reuses wq's slot; its loads overlap the tail of the QK pass
            wv_sb = wpool.tile([128, HC, NH * D], F32R, tag="w", name="wv")
            for c in range(4):
                h4 = slice(4 * c, 4 * c + 4)
                nc.sync.dma_start(wv_sb[:, h4, :], wvT_r[:, h4, :])

        # ================= Phase A2: V projection =================
        # v_sb reuses wk's slot; natural orientation, resident through B
        v_sb = wpool.tile([128, S // 128, NH * D], F32R, tag="w", name="vsb")
        with ExitStack() as a2ctx:
            h2pool = a2ctx.enter_context(tc.tile_pool(name="ah2", bufs=4))
            psAv = a2ctx.enter_context(
                tc.tile_pool(name="apsv", bufs=3, space="PSUM"))
            for st in range(NST):
                sl = bass.ts(st, ST)
                hq = [h2pool.tile([128, 4, ST], F32R, tag="h2",
                                  name=f"hq{q}") for q in range(4)]
                for q in range(4):
                    nc.sync.dma_start(hq[q][:], hT_r[:, 4 * q:4 * q + 4, sl])
                for sc in range(ST // 128):
                    ps = psAv.tile([128, NH * D], F32, tag="psv", name="psv")
                    for hc in range(HC):
                        nc.tensor.matmul(
                            ps[:],
                            hq[hc // 4][:, hc % 4, sc * 128:(sc + 1) * 128],
                            wv_sb[:, hc, :],
                            start=(hc == 0), stop=(hc == HC - 1),
                        )
                    nc.scalar.copy(
                        v_sb[:, st * (ST // 128) + sc, :], ps[:])

        # A^T (phase B -> C) reuses wv's slot; w_o prefetches during B
        at_all = wpool.tile([128, NH, S], F32R, tag="w", name="at")
        wop = ctx.enter_context(tc.tile_pool(name="cwo", bufs=1))
        wo_sb = wop.tile([128, NH, HID], F32R, tag="wo", name="wo")
        for g in range(NH):
            nc.sync.dma_start(wo_sb[:, g, :], woT_r[:, g, :])

        # ================= Phase B =================
        with ExitStack() as bctx:
            expp = bctx.enter_context(tc.tile_pool(name="bexp", bufs=6))
            smallp = bctx.enter_context(tc.tile_pool(name="bsmall", bufs=3))
            psS = bctx.enter_context(
                tc.tile_pool(name="bpss", bufs=2, space="PSUM"))
            psPV = bctx.enter_context(
                tc.tile_pool(name="bpspv", bufs=2, space="PSUM"))
            psCS = bctx.enter_context(
                tc.tile_pool(name="bpscs", bufs=1, space="PSUM"))
            psRB = bctx.enter_context(
                tc.tile_pool(name="bpsrb", bufs=1, space="PSUM"))

            for h in range(NH):
                for qt in range(NQT):
                    nallow = (QT // 128) * qt + (QT // 128)
                    qsl = bass.ts(qt, QT)
                    pvps = psPV.tile([128, QT], F32, tag="pv", name="pv")
                    csps = psCS.tile([1, QT], F32, tag="cs", name="cs")

                    # scores^T in 2-chunk PSUM groups; exp to SBUF groups
                    ngrp = (nallow + 1) // 2
                    egrp = []
                    for g in range(ngrp):
                        k0 = 2 * g
                        nk = min(2, nallow - k0)
                        sps = psS.tile([128, 2, QT], F32, tag="s", name="s")
                        eb = expp.tile([128, 2, QT], F32R, tag="e", name="e")
                        egrp.append(eb)
                        for i in range(nk):
                            kc = k0 + i
                            nc.tensor.matmul(
                                sps[:, i, :],
                                ksb[:, h, kc * 128:(kc + 1) * 128],
                                qsb[:, h, qsl],
                                start=True, stop=True,
                            )
                        j0 = k0 - 4 * qt
                        if j0 + nk - 1 < 0:
                            nc.scalar.activation(
                                eb[:, 0:nk, :], sps[:, 0:nk, :],
                                mybir.ActivationFunctionType.Exp,
                                scale=DSCALE)
                        else:
                            for i in range(nk):
                                kc = k0 + i
                                j = kc - 4 * qt
                                lo = max(0, 128 * j)
                                nc.scalar.activation(
                                    eb[:, i, lo:QT], sps[:, i, lo:QT],
                                    mybir.ActivationFunctionType.Exp,
                                    scale=DSCALE)
                                if j >= 0:
                                    nc.vector.tensor_tensor(
                                        eb[:, i, lo:lo + 128],
                                        eb[:, i, lo:lo + 128].bitcast(F32),
                                        tri_sb[:],
                                        mybir.AluOpType.mult)

                    # colsum + PV accumulation over allowed chunks
                    for kc in range(nallow):
                        j = kc - 4 * qt
                        lo = max(0, 128 * j)
                        eb = egrp[kc // 2]
                        i = kc % 2
                        nc.tensor.matmul(
                            csps[:, lo:QT], ones_sb[:],
                            eb[:, i, lo:QT],
                            start=(kc == 0), stop=(kc == nallow - 1),
                            skip_group_check=True,
                        )
                        nc.tensor.matmul(
                            pvps[:, lo:QT],
                            v_sb[:, kc, h * D:(h + 1) * D],
                            eb[:, i, lo:QT],
                            start=(kc == 0), stop=(kc == nallow - 1),
                            skip_group_check=True,
                        )

                    # normalize: at = pv * broadcast(1/colsum). Broadcast
                    # via a K=1 PE matmul (ones column x reciprocal row).
                    rec = smallp.tile([1, QT], F32R, tag="rec", name="rec")
                    with nc.allow_low_precision(
                            reason="softmax denom reciprocal to f32r"):
                        nc.vector.reciprocal(rec[:], csps[:])
                    rbc = psRB.tile([128, QT], F32, tag="rbc", name="rbc")
                    nc.tensor.matmul(rbc[:], onesr_sb[:], rec[:],
                                     start=True, stop=True)
                    at_t = smallp.tile([128, QT], F32, tag="att", name="att")
                    nc.vector.tensor_copy(at_t[:], pvps[:])
                    nc.vector.tensor_tensor(
                        at_all[:, h, qsl], at_t[:], rbc[:],
                        mybir.AluOpType.mult)

        # ================= Phase C =================
        with ExitStack() as cctx:
            outp = cctx.enter_context(tc.tile_pool(name="cout", bufs=3))
            psO = cctx.enter_context(
                tc.tile_pool(name="cpso", bufs=4, space="PSUM"))

            for sc in range(S // 128):
                ssl = bass.ts(sc, 128)
                ot = outp.tile([128, HID], F32, tag="ot", name="ot")
                for nt in range(HID // QT):
                    nsl = bass.ts(nt, QT)
                    ps = psO.tile([128, QT], F32, tag="o", name="o")
                    for g in range(NH):
                        nc.tensor.matmul(
                            ps[:],
                            at_all[:, g, ssl],
                            wo_sb[:, g, nsl],
                            start=(g == 0), stop=(g == NH - 1),
                        )
                    if nt % 2 == 0:
                        nc.vector.tensor_copy(ot[:, nsl], ps[:])
                    else:
                        nc.scalar.copy(ot[:, nsl], ps[:])
                nc.sync.dma_start(out.ap()[ssl, :], ot[:])

    nc.compile()
    return nc


def _prep_in_maps(hidden_states, cos, sin, w_qkv, w_o):
    hs = np.ascontiguousarray(np.asarray(hidden_states, dtype=np.float32))
    cos = np.asarray(cos, dtype=np.float32)
    sin = np.asarray(sin, dtype=np.float32)
    w_qkv = np.asarray(w_qkv, dtype=np.float32)
    w_o = np.asarray(w_o, dtype=np.float32)

    wT = np.ascontiguousarray(w_qkv.T)          # (HID, 3*H*D)
    woTf = np.ascontiguousarray(w_o.T)          # (H*D, HID)
    cosT = np.ascontiguousarray(cos.T)          # (D, S)
    sinT = np.ascontiguousarray(sin.T)
    sinS = sinT.copy()
    sinS[:64] = -sinT[:64]
    tri = np.triu(np.ones((128, 128), np.float32))
    ones = np.ones((128, 1), np.float32)

    hT = [np.ascontiguousarray(hs[b].T) for b in range(B)]

    in_maps = []
    for c in range(NCORES):
        b, hg = c // 4, c % 4
        lo, hi = hg * NH * D, (hg + 1) * NH * D
        in_maps.append({
            "hT": hT[b],
            "wqT": np.ascontiguousarray(wT[:, lo:hi]),
            "wkT": np.ascontiguousarray(wT[:, H * D + lo:H * D + hi]),
            "wvT": np.ascontiguousarray(wT[:, 2 * H * D + lo:2 * H * D + hi]),
            "woT": np.ascontiguousarray(woTf[lo:hi, :]),
            "cosT": cosT,
            "sinS": sinS,
            "tri": tri,
            "ones": ones,
            "onesr": np.ones((1, 128), np.float32),
        })
    return in_maps


def kernel(hidden_states, cos, sin, w_qkv, w_o, _trace=False):
    if "nc" not in _CACHED:
        _CACHED["nc"] = _build_nc()
    nc = _CACHED["nc"]
    in_maps = _prep_in_maps(hidden_states, cos, sin, w_qkv, w_o)
    res = run_bass_kernel_spmd(nc, in_maps, core_ids=list(range(NCORES)),
                               trace=_trace)
    _CACHED["last_result"] = res
    out = np.zeros((B, S, HID), np.float32)
    for c in range(NCORES):
        out[c // 4] += res.results[c]["out"]
    return out


# revision 21
# speedup vs baseline: 1.0041x; 1.0041x over previous
"""Trainium2 Bass kernel for AttentionWithRoPE (B=2, S=2048, HID=2048, H=16, D=128).

Sharding (8 cores): tensor-parallel over heads x data-parallel over batch.
Core c handles batch c//4 and heads 4*(c%4) .. 4*(c%4)+4:
  - QKV^T projections as fp32r PE matmuls with the HID contraction on
    partitions (hidden arrives pre-transposed from the host). Q pass, K pass,
    then V pass; Q^T/K^T/V stay resident in SBUF (a 2-slot rotating pool
    hosts wq/wk -> wv -> A^T over time, so each phase's loads prefetch
    during the previous one). RoPE is fused on the DVE reading the
    projection PSUM directly (partition-shifted reads are legal vs PSUM).
  - Causal attention per head in scores^T orientation ([k, q]: the PV
    contraction dim k lands on partitions, so P^T feeds the PE directly and
    softmax needs no transposes). exp on ScalarE straight from PSUM with the
    1/sqrt(D) scale folded in; fully-masked k-blocks are skipped; diagonal
    blocks multiply a 0/1 upper-tri mask; the softmax denominator is a
    ones-vector PE matmul accumulated alongside PV; normalization is
    reciprocal + K=1 PE-matmul broadcast + DVE multiply.
  - Output projection partial with the core's w_o column slice; the host
    sums the four partials per batch (the TP reduce).
All matmul operands are float32r (TF32-like: full PE rate at moving
free-dim >= 256, ~1.5e-4 rel err); end-to-end output error vs the fp32
reference is ~2e-4. Big DMAs are chunked so consumers start on the first
chunk; small DMAs are merged to amortize descriptor cost.
"""
import numpy as np
from contextlib import ExitStack

import concourse.bass as bass
import concourse.tile as tile
from concourse import bacc, mybir
from concourse.bass_utils import run_bass_kernel_spmd

B, S, HID = 2, 2048, 2048
H, D = 16, 128
NCORES = 8
NH = 4                 # heads per core
HC = HID // 128        # hid chunks
ST = 256               # phase-A s-tile width
NST = S // ST
QT = 512               # phase-B q-tile width
NQT = S // QT
DSCALE = float(D) ** -0.5
F32 = mybir.dt.float32
F32R = mybir.dt.float32r

_CACHED = {}


def _build_nc():
    nc = bacc.Bacc("TRN2", target_bir_lowering=False, debug=False,
                   num_devices=NCORES)
    hT = nc.dram_tensor("hT", [HID, S], F32R, kind="ExternalInput")
    wqT = nc.dram_tensor("wqT", [HID, NH * D], F32R, kind="ExternalInput")
    wkT = nc.dram_tensor("wkT", [HID, NH * D], F32R, kind="ExternalInput")
    wvT = nc.dram_tensor("wvT", [HID, NH * D], F32R, kind="ExternalInput")
    woT = nc.dram_tensor("woT", [NH * D, HID], F32R, kind="ExternalInput")
    cosT = nc.dram_tensor("cosT", [D, S], F32, kind="ExternalInput")
    sinS = nc.dram_tensor("sinS", [D, S], F32, kind="ExternalInput")
    tri = nc.dram_tensor("tri", [128, 128], F32, kind="ExternalInput")
    ones = nc.dram_tensor("ones", [128, 1], F32R, kind="ExternalInput")
    onesr = nc.dram_tensor("onesr", [1, 128], F32R, kind="ExternalInput")
    out = nc.dram_tensor("out", [S, HID], F32, kind="ExternalOutput")

    hT_r = hT.ap().rearrange("(hc p) s -> p hc s", p=128)
    wqT_r = wqT.ap().rearrange("(hc p) m -> p hc m", p=128)
    wkT_r = wkT.ap().rearrange("(hc p) m -> p hc m", p=128)
    wvT_r = wvT.ap().rearrange("(hc p) m -> p hc m", p=128)
    woT_r = woT.ap().rearrange("(g p) n -> p g n", p=128)

    AST = 512              # phase-A s-tile width (N of the QK-pass matmuls)
    ANST = S // AST

    with tile.TileContext(nc) as tc, ExitStack() as ctx:
        # ---- small constants ----
        constp = ctx.enter_context(tc.tile_pool(name="const", bufs=1))
        tri_sb = constp.tile([128, 128], F32, tag="tri", name="tri")
        nc.sync.dma_start(tri_sb[:], tri.ap())
        ones_sb = constp.tile([128, 1], F32R, tag="ones", name="ones")
        nc.sync.dma_start(ones_sb[:], ones.ap())
        onesr_sb = constp.tile([1, 128], F32R, tag="onesr", name="onesr")
        nc.sync.dma_start(onesr_sb[:], onesr.ap())

        # Q^T/K^T stay resident in SBUF through attention
        qkp = ctx.enter_context(tc.tile_pool(name="qk", bufs=1))
        qsb = qkp.tile([128, NH, S], F32R, tag="qsb", name="qsb")
        ksb = qkp.tile([128, NH, S], F32R, tag="ksb", name="ksb")

        # Rotating 2-slot pool (32KB/partition each) hosting, over time:
        #   slot0: wq -> wv -> at   slot1: wk -> v_sb
        # Tile's WAR tracking turns each reuse into a prefetch window.
        wpool = ctx.enter_context(tc.tile_pool(name="aw", bufs=2))
        wq_sb = wpool.tile([128, HC, NH * D], F32R, tag="w", name="wq")
        wk_sb = wpool.tile([128, HC, NH * D], F32R, tag="w", name="wk")
        for c in range(4):
            h4 = slice(4 * c, 4 * c + 4)
            nc.sync.dma_start(wq_sb[:, h4, :], wqT_r[:, h4, :])
        for c in range(4):
            h4 = slice(4 * c, 4 * c + 4)
            nc.sync.dma_start(wk_sb[:, h4, :], wkT_r[:, h4, :])

        # ================= Phase A: Q pass, K pass =================
        with ExitStack() as astack:
            hpool = astack.enter_context(tc.tile_pool(name="ah", bufs=3))
            cspool = astack.enter_context(tc.tile_pool(name="acs", bufs=1))
            ropep = astack.enter_context(tc.tile_pool(name="arope", bufs=1))
            psA = astack.enter_context(
                tc.tile_pool(name="apsqk", bufs=3, space="PSUM"))

            for st in range(ANST):
                sl = bass.ts(st, AST)
                hb = [hpool.tile([128, HC // 2, AST], F32R, tag="h",
                                 name=f"hb{half}") for half in range(2)]
                for half in range(2):
                    for c in range(4):
                        nc.sync.dma_start(
                            hb[half][:, 2 * c:2 * c + 2, :],
                            hT_r[:, slice(8 * half + 2 * c,
                                          8 * half + 2 * c + 2), sl])
                cs_t = cspool.tile([128, AST], F32, tag="cs", name="cs")
                nc.sync.dma_start(cs_t[:], cosT.ap()[:, sl])
                ss_t = cspool.tile([128, AST], F32, tag="ss", name="ss")
                nc.sync.dma_start(ss_t[:], sinS.ap()[:, sl])
                for wsb, dsb in ((wq_sb, qsb), (wk_sb, ksb)):
                    for h in range(NH):
                        ps = psA.tile([128, AST], F32, tag="psqk",
                                      name="psqk")
                        for hc in range(HC):
                            nc.tensor.matmul(
                                ps[:],
                                wsb[:, hc, h * D:(h + 1) * D],
                                hb[hc // 8][:, hc % 8, :],
                                start=(hc == 0), stop=(hc == HC - 1),
                            )
                        # RoPE: out = x*cos + shift(x)*sin_signed. The
                        # partition-shifted reads go straight to PSUM (walrus
                        # requires equal base partitions only when BOTH
                        # operands are in SBUF).
                        tsin = ropep.tile([128, AST], F32, tag="tsin",
                                          name="tsin")
                        nc.vector.tensor_tensor(
                            tsin[0:64, :], ps[64:128, :], ss_t[0:64, :],
                            mybir.AluOpType.mult)
                        nc.vector.tensor_tensor(
                            tsin[64:128, :], ps[0:64, :], ss_t[64:128, :],
                            mybir.AluOpType.mult)
                        tcos = ropep.tile([128, AST], F32, tag="tcos",
                                          name="tcos")
                        nc.vector.tensor_tensor(
                            tcos[:], ps[:], cs_t[:], mybir.AluOpType.mult)
                        nc.vector.tensor_tensor(
                            dsb[:, h, sl], tcos[:], tsin[:],
                            mybir.AluOpType.add)

            # wv reuses wq's slot; its loads overlap the tail of the QK pass
            wv_sb = wpool.tile([128, HC, NH * D], F32R, tag="w", name="wv")
            for c in range(4):
                h4 = slice(4 * c, 4 * c + 4)
                nc.sync.dma_start(wv_sb[:, h4, :], wvT_r[:, h4, :])

        # ================= Phase A2: V projection =================
        # v_sb reuses wk's slot; natural orientation, resident through B
        v_sb = wpool.tile([128, S // 128, NH * D], F32R, tag="w", name="vsb")
        with ExitStack() as a2ctx:
            h2pool = a2ctx.enter_context(tc.tile_pool(name="ah2", bufs=4))
            psAv = a2ctx.enter_context(
                tc.tile_pool(name="apsv", bufs=3, space="PSUM"))
            for st in range(NST):
                sl = bass.ts(st, ST)
                hq = [h2pool.tile([128, 4, ST], F32R, tag="h2",
                                  name=f"hq{q}") for q in range(4)]
                for q in range(4):
                    nc.sync.dma_start(hq[q][:], hT_r[:, 4 * q:4 * q + 4, sl])
                for sc in range(ST // 128):
                    ps = psAv.tile([128, NH * D], F32, tag="psv", name="psv")
                    for hc in range(HC):
                        nc.tensor.matmul(
                            ps[:],
                            hq[hc // 4][:, hc % 4, sc * 128:(sc + 1) * 128],
                            wv_sb[:, hc, :],
                            start=(hc == 0), stop=(hc == HC - 1),
                        )
                    nc.scalar.copy(
                        v_sb[:, st * (ST // 128) + sc, :], ps[:])

        # A^T (phase B -> C) reuses wv's slot; w_o prefetches during B
        at_all = wpool.tile([128, NH, S], F32R, tag="w", name="at")
        wop = ctx.enter_context(tc.tile_pool(name="cwo", bufs=1))
        wo_sb = wop.tile([128, NH, HID], F32R, tag="wo", name="wo")
        for g in range(NH):
            nc.sync.dma_start(wo_sb[:, g, :], woT_r[:, g, :])

        # ================= Phase B =================
        with ExitStack() as bctx:
            expp = bctx.enter_context(tc.tile_pool(name="bexp", bufs=6))
            smallp = bctx.enter_context(tc.tile_pool(name="bsmall", bufs=3))
            psS = bctx.enter_context(
                tc.tile_pool(name="bpss", bufs=2, space="PSUM"))
            psPV = bctx.enter_context(
                tc.tile_pool(name="bpspv", bufs=2, space="PSUM"))
            psCS = bctx.enter_context(
                tc.tile_pool(name="bpscs", bufs=2, space="PSUM"))

            for h in range(NH):
                for qt in range(NQT):
                    nallow = (QT // 128) * qt + (QT // 128)
                    qsl = bass.ts(qt, QT)
                    pvps = psPV.tile([128, QT], F32, tag="pv", name="pv")
                    csps = psCS.tile([1, QT], F32, tag="cs", name="cs")

                    # scores^T in 2-chunk PSUM groups; exp to SBUF groups
                    ngrp = (nallow + 1) // 2
                    egrp = []
                    for g in range(ngrp):
                        k0 = 2 * g
                        nk = min(2, nallow - k0)
                        sps = psS.tile([128, 2, QT], F32, tag="s", name="s")
                        eb = expp.tile([128, 2, QT], F32R, tag="e", name="e")
                        egrp.append(eb)
                        for i in range(nk):
                            kc = k0 + i
                            lo = max(0, 128 * (kc - 4 * qt))
                            nc.tensor.matmul(
                                sps[:, i, lo:QT],
                                ksb[:, h, kc * 128:(kc + 1) * 128],
                                qsb[:, h, qt * QT + lo:(qt + 1) * QT],
                                start=True, stop=True,
                            )
                        j0 = k0 - 4 * qt
                        if j0 + nk - 1 < 0:
                            nc.scalar.activation(
                                eb[:, 0:nk, :], sps[:, 0:nk, :],
                                mybir.ActivationFunctionType.Exp,
                                scale=DSCALE)
                        else:
                            for i in range(nk):
                                kc = k0 + i
                                j = kc - 4 * qt
                                lo = max(0, 128 * j)
                                nc.scalar.activation(
                                    eb[:, i, lo:QT], sps[:, i, lo:QT],
                                    mybir.ActivationFunctionType.Exp,
                                    scale=DSCALE)
                                if j >= 0:
                                    nc.vector.tensor_tensor(
                                        eb[:, i, lo:lo + 128],
                                        eb[:, i, lo:lo + 128].bitcast(F32),
                                        tri_sb[:],
                                        mybir.AluOpType.mult)

                    # colsum + PV accumulation over allowed chunks
                    for kc in range(nallow):
                        j = kc - 4 * qt
                        lo = max(0, 128 * j)
                        eb = egrp[kc // 2]
                        i = kc % 2
                        nc.tensor.matmul(
                            csps[:, lo:QT], ones_sb[:],
                            eb[:, i, lo:QT],
                            start=(kc == 0), stop=(kc == nallow - 1),
                            skip_group_check=True,
                        )
                        nc.tensor.matmul(
                            pvps[:, lo:QT],
                            v_sb[:, kc, h * D:(h + 1) * D],
                            eb[:, i, lo:QT],
                            start=(kc == 0), stop=(kc == nallow - 1),
                            skip_group_check=True,
                        )

                    # normalize: at = pv * broadcast(1/colsum). Broadcast
                    # via a K=1 PE matmul (ones column x reciprocal row).
                    rec = smallp.tile([1, QT], F32R, tag="rec", name="rec")
                    with nc.allow_low_precision(
                            reason="softmax denom reciprocal to f32r"):
                        nc.vector.reciprocal(rec[:], csps[:])
                    rbc = psPV.tile([128, QT], F32, tag="pv", name="rbc")
                    nc.tensor.matmul(rbc[:], onesr_sb[:], rec[:],
                                     start=True, stop=True)
                    at_t = smallp.tile([128, QT], F32, tag="att", name="att")
                    nc.vector.tensor_copy(at_t[:], pvps[:])
                    nc.vector.tensor_tensor(
                        at_all[:, h, qsl], at_t[:], rbc[:],
                        mybir.AluOpType.mult)

        # ================= Phase C =================
        with ExitStack() as cctx:
            outp = cctx.enter_context(tc.tile_pool(name="cout", bufs=3))
            psO = cctx.enter_context(
                tc.tile_pool(name="cpso", bufs=4, space="PSUM"))

            for sc in range(S // 128):
                ssl = bass.ts(sc, 128)
                ot = outp.tile([128, HID], F32, tag="ot", name="ot")
                for nt in range(HID // QT):
                    nsl = bass.ts(nt, QT)
                    ps = psO.tile([128, QT], F32, tag="o", name="o")
                    for g in range(NH):
                        nc.tensor.matmul(
                            ps[:],
                            at_all[:, g, ssl],
                            wo_sb[:, g, nsl],
                            start=(g == 0), stop=(g == NH - 1),
                        )
                    if nt % 2 == 0:
                        nc.vector.tensor_copy(ot[:, nsl], ps[:])
                    else:
                        nc.scalar.copy(ot[:, nsl], ps[:])
                nc.sync.dma_start(out.ap()[ssl, :], ot[:])

    nc.compile()
    return nc


def _prep_in_maps(hidden_states, cos, sin, w_qkv, w_o):
    hs = np.ascontiguousarray(np.asarray(hidden_states, dtype=np.float32))
    cos = np.asarray(cos, dtype=np.float32)
    sin = np.asarray(sin, dtype=np.float32)
    w_qkv = np.asarray(w_qkv, dtype=np.float32)
    w_o = np.asarray(w_o, dtype=np.float32)

    wT = np.ascontiguousarray(w_qkv.T)          # (HID, 3*H*D)
    woTf = np.ascontiguousarray(w_o.T)          # (H*D, HID)
    cosT = np.ascontiguousarray(cos.T)          # (D, S)
    sinT = np.ascontiguousarray(sin.T)
    sinS = sinT.copy()
    sinS[:64] = -sinT[:64]
    tri = np.triu(np.ones((128, 128), np.float32))
    ones = np.ones((128, 1), np.float32)

    hT = [np.ascontiguousarray(hs[b].T) for b in range(B)]

    in_maps = []
    for c in range(NCORES):
        b, hg = c // 4, c % 4
        lo, hi = hg * NH * D, (hg + 1) * NH * D
        in_maps.append({
            "hT": hT[b],
            "wqT": np.ascontiguousarray(wT[:, lo:hi]),
            "wkT": np.ascontiguousarray(wT[:, H * D + lo:H * D + hi]),
            "wvT": np.ascontiguousarray(wT[:, 2 * H * D + lo:2 * H * D + hi]),
            "woT": np.ascontiguousarray(woTf[lo:hi, :]),
            "cosT": cosT,
            "sinS": sinS,
            "tri": tri,
            "ones": ones,
            "onesr": np.ones((1, 128), np.float32),
        })
    return in_maps


def kernel(hidden_states, cos, sin, w_qkv, w_o, _trace=False):
    if "nc" not in _CACHED:
        _CACHED["nc"] = _build_nc()
    nc = _CACHED["nc"]
    in_maps = _prep_in_maps(hidden_states, cos, sin, w_qkv, w_o)
    res = run_bass_kernel_spmd(nc, in_maps, core_ids=list(range(NCORES)),
                               trace=_trace)
    _CACHED["last_result"] = res
    out = np.zeros((B, S, HID), np.float32)
    for c in range(NCORES):
        out[c // 4] += res.results[c]["out"]
    return out


# revision 22
# speedup vs baseline: 1.0214x; 1.0172x over previous
"""Trainium2 Bass kernel for AttentionWithRoPE (B=2, S=2048, HID=2048, H=16, D=128).

Sharding (8 cores): tensor-parallel over heads x data-parallel over batch.
Core c handles batch c//4 and heads 4*(c%4) .. 4*(c%4)+4:
  - QKV^T projections as fp32r PE matmuls with the HID contraction on
    partitions (hidden arrives pre-transposed from the host). Q pass, K pass,
    then V pass; Q^T/K^T/V stay resident in SBUF (a 2-slot rotating pool
    hosts wq/wk -> wv -> A^T over time, so each phase's loads prefetch
    during the previous one). RoPE is fused on the DVE reading the
    projection PSUM directly (partition-shifted reads are legal vs PSUM).
  - Causal attention per head in scores^T orientation ([k, q]: the PV
    contraction dim k lands on partitions, so P^T feeds the PE directly and
    softmax needs no transposes). exp on ScalarE straight from PSUM with the
    1/sqrt(D) scale folded in; fully-masked k-blocks are skipped; diagonal
    blocks multiply a 0/1 upper-tri mask; the softmax denominator is a
    ones-vector PE matmul accumulated alongside PV; normalization is
    reciprocal + K=1 PE-matmul broadcast + DVE multiply.
  - Output projection partial with the core's w_o column slice; the host
    sums the four partials per batch (the TP reduce).
All matmul operands are float32r (TF32-like: full PE rate at moving
free-dim >= 256, ~1.5e-4 rel err); end-to-end output error vs the fp32
reference is ~2e-4. Big DMAs are chunked so consumers start on the first
chunk; small DMAs are merged to amortize descriptor cost.
"""
import numpy as np
from contextlib import ExitStack

import concourse.bass as bass
import concourse.tile as tile
from concourse import bacc, mybir
from concourse.bass_utils import run_bass_kernel_spmd

B, S, HID = 2, 2048, 2048
H, D = 16, 128
NCORES = 8
NH = 4                 # heads per core
HC = HID // 128        # hid chunks
ST = 256               # phase-A s-tile width
NST = S // ST
QT = 512               # phase-B q-tile width
NQT = S // QT
DSCALE = float(D) ** -0.5
F32 = mybir.dt.float32
F32R = mybir.dt.float32r

_CACHED = {}


def _build_nc():
    nc = bacc.Bacc("TRN2", target_bir_lowering=False, debug=False,
                   num_devices=NCORES)
    hT = nc.dram_tensor("hT", [HID, S], F32R, kind="ExternalInput")
    wqT = nc.dram_tensor("wqT", [HID, NH * D], F32R, kind="ExternalInput")
    wkT = nc.dram_tensor("wkT", [HID, NH * D], F32R, kind="ExternalInput")
    wvT = nc.dram_tensor("wvT", [HID, NH * D], F32R, kind="ExternalInput")
    woT = nc.dram_tensor("woT", [NH * D, HID], F32R, kind="ExternalInput")
    cosT = nc.dram_tensor("cosT", [D, S], F32, kind="ExternalInput")
    sinS = nc.dram_tensor("sinS", [D, S], F32, kind="ExternalInput")
    tri = nc.dram_tensor("tri", [128, 128], F32, kind="ExternalInput")
    ones = nc.dram_tensor("ones", [128, 1], F32R, kind="ExternalInput")
    onesr = nc.dram_tensor("onesr", [1, 128], F32R, kind="ExternalInput")
    out = nc.dram_tensor("out", [S, HID], F32, kind="ExternalOutput")

    hT_r = hT.ap().rearrange("(hc p) s -> p hc s", p=128)
    wqT_r = wqT.ap().rearrange("(hc p) m -> p hc m", p=128)
    wkT_r = wkT.ap().rearrange("(hc p) m -> p hc m", p=128)
    wvT_r = wvT.ap().rearrange("(hc p) m -> p hc m", p=128)
    woT_r = woT.ap().rearrange("(g p) n -> p g n", p=128)

    AST = 512              # phase-A s-tile width (N of the QK-pass matmuls)
    ANST = S // AST

    with tile.TileContext(nc) as tc, ExitStack() as ctx:
        # ---- small constants ----
        constp = ctx.enter_context(tc.tile_pool(name="const", bufs=1))
        tri_sb = constp.tile([128, 128], F32, tag="tri", name="tri")
        nc.sync.dma_start(tri_sb[:], tri.ap())
        ones_sb = constp.tile([128, 1], F32R, tag="ones", name="ones")
        nc.sync.dma_start(ones_sb[:], ones.ap())
        onesr_sb = constp.tile([1, 128], F32R, tag="onesr", name="onesr")
        nc.sync.dma_start(onesr_sb[:], onesr.ap())

        # Q^T/K^T stay resident in SBUF through attention
        qkp = ctx.enter_context(tc.tile_pool(name="qk", bufs=1))
        qsb = qkp.tile([128, NH, S], F32R, tag="qsb", name="qsb")
        ksb = qkp.tile([128, NH, S], F32R, tag="ksb", name="ksb")

        # Rotating 2-slot pool (32KB/partition each) hosting, over time:
        #   slot0: wq -> wv -> at   slot1: wk -> v_sb
        # Tile's WAR tracking turns each reuse into a prefetch window.
        wpool = ctx.enter_context(tc.tile_pool(name="aw", bufs=2))
        wq_sb = wpool.tile([128, HC, NH * D], F32R, tag="w", name="wq")
        wk_sb = wpool.tile([128, HC, NH * D], F32R, tag="w", name="wk")
        for c in range(4):
            h4 = slice(4 * c, 4 * c + 4)
            nc.sync.dma_start(wq_sb[:, h4, :], wqT_r[:, h4, :])
        for c in range(4):
            h4 = slice(4 * c, 4 * c + 4)
            nc.sync.dma_start(wk_sb[:, h4, :], wkT_r[:, h4, :])

        # ================= Phase A: Q pass, K pass =================
        with ExitStack() as astack:
            hpool = astack.enter_context(tc.tile_pool(name="ah", bufs=3))
            cspool = astack.enter_context(tc.tile_pool(name="acs", bufs=2))
            ropep = astack.enter_context(tc.tile_pool(name="arope", bufs=1))
            psA = astack.enter_context(
                tc.tile_pool(name="apsqk", bufs=5, space="PSUM"))

            for st in range(ANST):
                sl = bass.ts(st, AST)
                hb = [hpool.tile([128, HC // 2, AST], F32R, tag="h",
                                 name=f"hb{half}") for half in range(2)]
                for half in range(2):
                    for c in range(4):
                        nc.sync.dma_start(
                            hb[half][:, 2 * c:2 * c + 2, :],
                            hT_r[:, slice(8 * half + 2 * c,
                                          8 * half + 2 * c + 2), sl])
                cs_t = cspool.tile([128, AST], F32, tag="cs", name="cs")
                nc.sync.dma_start(cs_t[:], cosT.ap()[:, sl])
                ss_t = cspool.tile([128, AST], F32, tag="ss", name="ss")
                nc.sync.dma_start(ss_t[:], sinS.ap()[:, sl])
                for wsb, dsb in ((wq_sb, qsb), (wk_sb, ksb)):
                    for h in range(NH):
                        ps = psA.tile([128, AST], F32, tag="psqk",
                                      name="psqk")
                        for hc in range(HC):
                            nc.tensor.matmul(
                                ps[:],
                                wsb[:, hc, h * D:(h + 1) * D],
                                hb[hc // 8][:, hc % 8, :],
                                start=(hc == 0), stop=(hc == HC - 1),
                            )
                        # RoPE: out = x*cos + shift(x)*sin_signed. The
                        # partition-shifted reads go straight to PSUM (walrus
                        # requires equal base partitions only when BOTH
                        # operands are in SBUF).
                        tsin = ropep.tile([128, AST], F32, tag="tsin",
                                          name="tsin")
                        nc.vector.tensor_tensor(
                            tsin[0:64, :], ps[64:128, :], ss_t[0:64, :],
                            mybir.AluOpType.mult)
                        nc.vector.tensor_tensor(
                            tsin[64:128, :], ps[0:64, :], ss_t[64:128, :],
                            mybir.AluOpType.mult)
                        tcos = ropep.tile([128, AST], F32, tag="tcos",
                                          name="tcos")
                        nc.vector.tensor_tensor(
                            tcos[:], ps[:], cs_t[:], mybir.AluOpType.mult)
                        nc.vector.tensor_tensor(
                            dsb[:, h, sl], tcos[:], tsin[:],
                            mybir.AluOpType.add)

            # wv reuses wq's slot; its loads overlap the tail of the QK pass
            wv_sb = wpool.tile([128, HC, NH * D], F32R, tag="w", name="wv")
            for c in range(4):
                h4 = slice(4 * c, 4 * c + 4)
                nc.sync.dma_start(wv_sb[:, h4, :], wvT_r[:, h4, :])

        # ================= Phase A2: V projection =================
        # v_sb reuses wk's slot; natural orientation, resident through B
        v_sb = wpool.tile([128, S // 128, NH * D], F32R, tag="w", name="vsb")
        with ExitStack() as a2ctx:
            h2pool = a2ctx.enter_context(tc.tile_pool(name="ah2", bufs=4))
            psAv = a2ctx.enter_context(
                tc.tile_pool(name="apsv", bufs=3, space="PSUM"))
            for st in range(NST):
                sl = bass.ts(st, ST)
                hq = [h2pool.tile([128, 4, ST], F32R, tag="h2",
                                  name=f"hq{q}") for q in range(4)]
                for q in range(4):
                    nc.sync.dma_start(hq[q][:], hT_r[:, 4 * q:4 * q + 4, sl])
                for sc in range(ST // 128):
                    ps = psAv.tile([128, NH * D], F32, tag="psv", name="psv")
                    for hc in range(HC):
                        nc.tensor.matmul(
                            ps[:],
                            hq[hc // 4][:, hc % 4, sc * 128:(sc + 1) * 128],
                            wv_sb[:, hc, :],
                            start=(hc == 0), stop=(hc == HC - 1),
                        )
                    nc.scalar.copy(
                        v_sb[:, st * (ST // 128) + sc, :], ps[:])

        # A^T (phase B -> C) reuses wv's slot; w_o prefetches during B
        at_all = wpool.tile([128, NH, S], F32R, tag="w", name="at")
        wop = ctx.enter_context(tc.tile_pool(name="cwo", bufs=1))
        wo_sb = wop.tile([128, NH, HID], F32R, tag="wo", name="wo")
        for g in range(NH):
            nc.sync.dma_start(wo_sb[:, g, :], woT_r[:, g, :])

        # ================= Phase B =================
        with ExitStack() as bctx:
            expp = bctx.enter_context(tc.tile_pool(name="bexp", bufs=6))
            smallp = bctx.enter_context(tc.tile_pool(name="bsmall", bufs=3))
            psS = bctx.enter_context(
                tc.tile_pool(name="bpss", bufs=2, space="PSUM"))
            psPV = bctx.enter_context(
                tc.tile_pool(name="bpspv", bufs=2, space="PSUM"))
            psCS = bctx.enter_context(
                tc.tile_pool(name="bpscs", bufs=2, space="PSUM"))

            for h in range(NH):
                for qt in range(NQT):
                    nallow = (QT // 128) * qt + (QT // 128)
                    qsl = bass.ts(qt, QT)
                    pvps = psPV.tile([128, QT], F32, tag="pv", name="pv")
                    csps = psCS.tile([1, QT], F32, tag="cs", name="cs")

                    # scores^T in 2-chunk PSUM groups; exp to SBUF groups
                    ngrp = (nallow + 1) // 2
                    egrp = []
                    for g in range(ngrp):
                        k0 = 2 * g
                        nk = min(2, nallow - k0)
                        sps = psS.tile([128, 2, QT], F32, tag="s", name="s")
                        eb = expp.tile([128, 2, QT], F32R, tag="e", name="e")
                        egrp.append(eb)
                        for i in range(nk):
                            kc = k0 + i
                            lo = max(0, 128 * (kc - 4 * qt))
                            nc.tensor.matmul(
                                sps[:, i, lo:QT],
                                ksb[:, h, kc * 128:(kc + 1) * 128],
                                qsb[:, h, qt * QT + lo:(qt + 1) * QT],
                                start=True, stop=True,
                            )
                        j0 = k0 - 4 * qt
                        if j0 + nk - 1 < 0:
                            nc.scalar.activation(
                                eb[:, 0:nk, :], sps[:, 0:nk, :],
                                mybir.ActivationFunctionType.Exp,
                                scale=DSCALE)
                        else:
                            for i in range(nk):
                                kc = k0 + i
                                j = kc - 4 * qt
                                lo = max(0, 128 * j)
                                nc.scalar.activation(
                                    eb[:, i, lo:QT], sps[:, i, lo:QT],
                                    mybir.ActivationFunctionType.Exp,
                                    scale=DSCALE)
                                if j >= 0:
                                    nc.vector.tensor_tensor(
                                        eb[:, i, lo:lo + 128],
                                        eb[:, i, lo:lo + 128].bitcast(F32),
                                        tri_sb[:],
                                        mybir.AluOpType.mult)

                    # colsum + PV accumulation over allowed chunks
                    for kc in range(nallow):
                        j = kc - 4 * qt
                        lo = max(0, 128 * j)
                        eb = egrp[kc // 2]
                        i = kc % 2
                        nc.tensor.matmul(
                            csps[:, lo:QT], ones_sb[:],
                            eb[:, i, lo:QT],
                            start=(kc == 0), stop=(kc == nallow - 1),
                            skip_group_check=True,
                        )
                        nc.tensor.matmul(
                            pvps[:, lo:QT],
                            v_sb[:, kc, h * D:(h + 1) * D],
                            eb[:, i, lo:QT],
                            start=(kc == 0), stop=(kc == nallow - 1),
                            skip_group_check=True,
                        )

                    # normalize: at = pv * broadcast(1/colsum). Broadcast
                    # via a K=1 PE matmul (ones column x reciprocal row).
                    rec = smallp.tile([1, QT], F32R, tag="rec", name="rec")
                    with nc.allow_low_precision(
                            reason="softmax denom reciprocal to f32r"):
                        nc.vector.reciprocal(rec[:], csps[:])
                    rbc = psPV.tile([128, QT], F32, tag="pv", name="rbc")
                    nc.tensor.matmul(rbc[:], onesr_sb[:], rec[:],
                                     start=True, stop=True)
                    at_t = smallp.tile([128, QT], F32, tag="att", name="att")
                    nc.vector.tensor_copy(at_t[:], pvps[:])
                    nc.vector.tensor_tensor(
                        at_all[:, h, qsl], at_t[:], rbc[:],
                        mybir.AluOpType.mult)

        # ================= Phase C =================
        with ExitStack() as cctx:
            outp = cctx.enter_context(tc.tile_pool(name="cout", bufs=3))
            psO = cctx.enter_context(
                tc.tile_pool(name="cpso", bufs=4, space="PSUM"))

            for sc in range(S // 128):
                ssl = bass.ts(sc, 128)
                ot = outp.tile([128, HID], F32, tag="ot", name="ot")
                for nt in range(HID // QT):
                    nsl = bass.ts(nt, QT)
                    ps = psO.tile([128, QT], F32, tag="o", name="o")
                    for g in range(NH):
                        nc.tensor.matmul(
                            ps[:],
                            at_all[:, g, ssl],
                            wo_sb[:, g, nsl],
                            start=(g == 0), stop=(g == NH - 1),
                        )
                    if nt % 2 == 0:
                        nc.vector.tensor_copy(ot[:, nsl], ps[:])
                    else:
                        nc.scalar.copy(ot[:, nsl], ps[:])
                nc.sync.dma_start(out.ap()[ssl, :], ot[:])

    nc.compile()
    return nc


def _prep_in_maps(hidden_states, cos, sin, w_qkv, w_o):
    hs = np.ascontiguousarray(np.asarray(hidden_states, dtype=np.float32))
    cos = np.asarray(cos, dtype=np.float32)
    sin = np.asarray(sin, dtype=np.float32)
    w_qkv = np.asarray(w_qkv, dtype=np.float32)
    w_o = np.asarray(w_o, dtype=np.float32)

    wT = np.ascontiguousarray(w_qkv.T)          # (HID, 3*H*D)
    woTf = np.ascontiguousarray(w_o.T)          # (H*D, HID)
    cosT = np.ascontiguousarray(cos.T)          # (D, S)
    sinT = np.ascontiguousarray(sin.T)
    sinS = sinT.copy()
    sinS[:64] = -sinT[:64]
    tri = np.triu(np.ones((128, 128), np.float32))
    ones = np.ones((128, 1), np.float32)

    hT = [np.ascontiguousarray(hs[b].T) for b in range(B)]

    in_maps = []
    for c in range(NCORES):
        b, hg = c // 4, c % 4
        lo, hi = hg * NH * D, (hg + 1) * NH * D
        in_maps.append({
            "hT": hT[b],
            "wqT": np.ascontiguousarray(wT[:, lo:hi]),
            "wkT": np.ascontiguousarray(wT[:, H * D + lo:H * D + hi]),
            "wvT": np.ascontiguousarray(wT[:, 2 * H * D + lo:2 * H * D + hi]),
            "woT": np.ascontiguousarray(woTf[lo:hi, :]),
            "cosT": cosT,
            "sinS": sinS,
            "tri": tri,
            "ones": ones,
            "onesr": np.ones((1, 128), np.float32),
        })
    return in_maps


def kernel(hidden_states, cos, sin, w_qkv, w_o, _trace=False):
    if "nc" not in _CACHED:
        _CACHED["nc"] = _build_nc()
    nc = _CACHED["nc"]
    in_maps = _prep_in_maps(hidden_states, cos, sin, w_qkv, w_o)
    res = run_bass_kernel_spmd(nc, in_maps, core_ids=list(range(NCORES)),
                               trace=_trace)
    _CACHED["last_result"] = res
    out = np.zeros((B, S, HID), np.float32)
    for c in range(NCORES):
        out[c // 4] += res.results[c]["out"]
    return out
